# revision 1
# baseline (speedup 1.0000x reference)
"""GraphSage 3-layer GNN on 8 Trainium2 NeuronCores.

Strategy: shard nodes (rows of A) across the 8 cores. The dominant cost
is streaming the dense adjacency (binary 0/1 matrix) from DRAM once per
layer. A is passed transposed (so the contraction dim lands on SBUF
partitions with contiguous DMA lines) and cast to bf16 on host -- exact
for a 0/1 matrix -- halving DRAM traffic. The mean-aggregation matmul
keeps A as the moving operand (1 cycle/row) with h-feature chunks as the
128x{128,32} stationary. Dense layers + l2norm + tanh run in the
transposed [feat, node] layout; an AllGather shares h between layers and
an AllReduce combines the global-sum-pool partials.
"""

import os
import sys
import types

import numpy as np

# ---------------------------------------------------------------- ntff hook
# The image lacks antenv.axon_hooks; inject it so trace=True (profiling,
# enabled via BASS_TRACE=1 by test.py) can capture NTFF under axon.
def _install_ntff_hook():
    if "antenv.axon_hooks" in sys.modules:
        return
    try:
        import antenv
        mod = types.ModuleType("antenv.axon_hooks")
        _hook = [None]
        mod.set_axon_ntff_profile_hook = lambda h: _hook.__setitem__(0, h)
        mod.get_axon_ntff_profile_hook = lambda: _hook[0]
        sys.modules["antenv.axon_hooks"] = mod
        antenv.axon_hooks = mod
        from trn_agent_boot.trn_boot import _ntff_profile_via_ctypes
        so = "/opt/axon/libaxon_pjrt.so"
        if os.path.exists(so):
            mod.set_axon_ntff_profile_hook(_ntff_profile_via_ctypes(so))
    except Exception:
        pass


_install_ntff_hook()

import ml_dtypes  # noqa: E402
import concourse.bass as bass  # noqa: E402
import concourse.bacc as bacc  # noqa: E402
import concourse.tile as tile  # noqa: E402
import concourse.mybir as mybir  # noqa: E402
from concourse.bass_utils import run_bass_kernel_spmd  # noqa: E402

# ------------------------------------------------------------------ geometry
N = 12000          # real nodes
F = 128            # input feature dim
H = 32             # hidden dim
NC = 8             # cores
NP = 12288         # padded nodes  (= 96*128 = 8*1536)
SH = NP // NC      # 1536 rows per core
KC = NP // 128     # 96 contraction chunks
MT = [(0, 512), (512, 512), (1024, 512)]   # m-tiles within the shard
NJ = SH // 128     # 12 transpose subtiles
TOL = 1e-6

MODE = os.environ.get("KMODE", "bf16")     # "bf16" | "f32r"
AT_BUFS = int(os.environ.get("KAT_BUFS", "16"))

F32 = mybir.dt.float32
if MODE == "bf16":
    DT_BIG = mybir.dt.bfloat16     # streamed A^T
    DT_STAT = mybir.dt.bfloat16    # stationary h chunks + allgathered h
    NP_BIG = ml_dtypes.bfloat16
    NP_STAT = ml_dtypes.bfloat16
else:
    DT_BIG = mybir.dt.float32r
    DT_STAT = mybir.dt.float32r
    NP_BIG = np.float32
    NP_STAT = np.float32

LAST_EXEC_NS = None
_CACHE = {}


# ------------------------------------------------------------------- builder
def _build():
    nc = bacc.Bacc("TRN2", target_bir_lowering=False, debug=False,
                   num_devices=NC)

    at_d = nc.dram_tensor("at", [NP, SH], DT_BIG, kind="ExternalInput")
    xs_d = nc.dram_tensor("xs", [NP, F], DT_STAT, kind="ExternalInput")
    xt_d = nc.dram_tensor("xt", [F, SH], F32, kind="ExternalInput")
    rc_d = nc.dram_tensor("rc", [F, SH], F32, kind="ExternalInput")
    w1t_d = nc.dram_tensor("w1t", [F, H], F32, kind="ExternalInput")
    w1b_d = nc.dram_tensor("w1b", [F, H], F32, kind="ExternalInput")
    w2t_d = nc.dram_tensor("w2t", [H, H], F32, kind="ExternalInput")
    w2b_d = nc.dram_tensor("w2b", [H, H], F32, kind="ExternalInput")
    w3t_d = nc.dram_tensor("w3t", [H, H], F32, kind="ExternalInput")
    w3b_d = nc.dram_tensor("w3b", [H, H], F32, kind="ExternalInput")
    wf1_d = nc.dram_tensor("wf1", [H, 2 * H], F32, kind="ExternalInput")
    wf2_d = nc.dram_tensor("wf2", [2 * H, 1], F32, kind="ExternalInput")
    b1_d = nc.dram_tensor("b1", [H, 1], F32, kind="ExternalInput")
    b2_d = nc.dram_tensor("b2", [H, 1], F32, kind="ExternalInput")
    b3_d = nc.dram_tensor("b3", [H, 1], F32, kind="ExternalInput")
    bf1_d = nc.dram_tensor("bf1", [2 * H, 1], F32, kind="ExternalInput")
    bf2_d = nc.dram_tensor("bf2", [1, 1], F32, kind="ExternalInput")
    i32_d = nc.dram_tensor("i32", [32, 32], F32, kind="ExternalInput")
    out_d = nc.dram_tensor("out", [1, 1], F32, kind="ExternalOutput")

    ag_in = [nc.dram_tensor(f"ag_in{l}", [SH, H], DT_STAT) for l in range(2)]
    ag_out = [nc.dram_tensor(f"ag_out{l}", [NP, H], DT_STAT,
                             addr_space="Shared") for l in range(2)]
    ar_in = nc.dram_tensor("ar_in", [H, 1], F32)
    ar_out = nc.dram_tensor("ar_out", [H, 1], F32, addr_space="Shared")
    rg = [list(range(NC))]

    with tile.TileContext(nc) as tc:
        with (
            tc.tile_pool(name="const", bufs=1) as constp,
            tc.tile_pool(name="xstat", bufs=1) as xstatp,
            tc.tile_pool(name="hstat", bufs=2) as hstatp,
            tc.tile_pool(name="hT", bufs=2) as hTp,
            tc.tile_pool(name="hnat", bufs=2) as hnatp,
            tc.tile_pool(name="at", bufs=AT_BUFS) as atp,
            tc.tile_pool(name="ep", bufs=4) as ep,
            tc.tile_pool(name="agg_ps", bufs=3, space=bass.MemorySpace.PSUM) as agg_ps,
            tc.tile_pool(name="z_ps", bufs=2, space=bass.MemorySpace.PSUM) as z_ps,
            tc.tile_pool(name="bc_ps", bufs=1, space=bass.MemorySpace.PSUM) as bc_ps,
            tc.tile_pool(name="t_ps", bufs=2, space=bass.MemorySpace.PSUM) as t_ps,
        ):
            # first x-stationary group before everything else on gpsimd
            xs = xstatp.tile([128, KC, F], DT_STAT)
            xs_r = xs_d.ap().rearrange("(k p) f -> p k f", p=128)
            nc.gpsimd.dma_start(xs[:, 0:8, :], xs_r[:, 0:8, :])

            # ---- constants
            def cload(dram, shape, dt=F32):
                t = constp.tile(shape, dt, tag=dram.name)
                nc.gpsimd.dma_start(t[:], dram[:, :])
                return t

            w1t = cload(w1t_d, [F, H])
            w1b = cload(w1b_d, [F, H])
            w2t = cload(w2t_d, [H, H])
            w2b = cload(w2b_d, [H, H])
            w3t = cload(w3t_d, [H, H])
            w3b = cload(w3b_d, [H, H])
            wf1 = cload(wf1_d, [H, 2 * H])
            wf2 = cload(wf2_d, [2 * H, 1])
            b1 = cload(b1_d, [H, 1])
            b2 = cload(b2_d, [H, 1])
            b3 = cload(b3_d, [H, 1])
            bf1 = cload(bf1_d, [2 * H, 1])
            bf2 = cload(bf2_d, [1, 1])
            i32 = cload(i32_d, [32, 32])
            rc = cload(rc_d, [F, SH])
            xt = cload(xt_d, [F, SH])

            ones_m = constp.tile([H, H], F32, tag="ones_m")
            nc.gpsimd.memset(ones_m[:], 1.0)

            # remaining x-stationary groups
            for g in range(8, KC, 8):
                nc.gpsimd.dma_start(xs[:, g:g + 8, :], xs_r[:, g:g + 8, :])

            def layer(li, fl, h_stat, hT, wtop, wbot, b):
                """one SageConv layer; returns hT_next [H, SH] fp32 tile."""
                hTn = hTp.tile([H, SH], F32, tag="hTn")
                # big aggregation matmul: k-outer, one wide DMA per
                # k-chunk (keeps the in-order sync DMA queue at 96 large
                # descriptors/layer), three 512-col matmuls accumulate into
                # three psum banks.
                paggs = []
                for _mi in range(len(MT)):
                    paggs.append(agg_ps.tile([fl, 512], F32, tag="pagg",
                                             name=f"pagg{li}_{_mi}"))
                for k in range(KC):
                    at_t = atp.tile([128, SH], DT_BIG, tag="at")
                    nc.sync.dma_start(
                        at_t[:], at_d[k * 128:(k + 1) * 128, :])
                    hk = h_stat[:, k, :] if h_stat is not None else xs[:, k, :]
                    for mi, (m0, mw) in enumerate(MT):
                        nc.tensor.matmul(
                            paggs[mi][:, :mw], hk, at_t[:, m0:m0 + mw],
                            start=(k == 0), stop=(k == KC - 1))
                zbs, sss = [], []
                for mi, (m0, mw) in enumerate(MT):
                    pagg = paggs[mi]
                    # scaled aggregation (mean): agg^T * (1/deg) broadcast
                    aggs = ep.tile([F, 512], F32, tag="aggs")
                    nc.vector.tensor_mul(
                        aggs[:fl, :mw], pagg[:fl, :mw], rc[:fl, m0:m0 + mw])
                    # z^T = Wtop^T h^T + Wbot^T agg^T
                    pz = z_ps.tile([H, 512], F32, tag="pz")
                    nc.tensor.matmul(pz[:, :mw], wtop[:, :], hT[:, m0:m0 + mw],
                                     start=True, stop=False)
                    nc.tensor.matmul(pz[:, :mw], wbot[:, :], aggs[:fl, :mw],
                                     start=False, stop=True)
                    zb = ep.tile([H, 512], F32, tag="zb")
                    nc.vector.tensor_scalar_add(zb[:, :mw], pz[:, :mw], b[:])
                    # row l2-norm over features (partition dim): sumsq via
                    # ones-matmul, broadcast back to H partitions, then do
                    # max/sqrt/recip at [H, mw] width (32 DVE lanes, and one
                    # activation-table load per layer instead of per tile).
                    sq = ep.tile([H, 512], F32, tag="sq")
                    nc.vector.tensor_mul(sq[:, :mw], zb[:, :mw], zb[:, :mw])
                    pbc = bc_ps.tile([H, 512], F32, tag="pbc")
                    nc.tensor.matmul(pbc[:, :mw], ones_m[:, :], sq[:, :mw],
                                     start=True, stop=True)
                    ssb = ep.tile([H, 512], F32, tag="ssb")
                    nc.vector.tensor_scalar_max(ssb[:, :mw], pbc[:, :mw], 1e-12)
                    zbs.append(zb)
                    sss.append(ssb)
                srts = []
                for mi, (m0, mw) in enumerate(MT):
                    srt = ep.tile([H, 512], F32, tag="srt")
                    nc.scalar.sqrt(srt[:, :mw], sss[mi][:, :mw])
                    srts.append(srt)
                if li < 2:
                    hnat = hnatp.tile([128, NJ, H], DT_STAT, tag="hnat",
                                      name=f"hnat{li}")
                else:
                    hnat = None
                for mi, (m0, mw) in enumerate(MT):
                    rn = ep.tile([H, 512], F32, tag="rn")
                    nc.vector.reciprocal_approx_fast(rn[:, :mw],
                                                     srts[mi][:, :mw])
                    zn = ep.tile([H, 512], F32, tag="zn")
                    nc.vector.tensor_mul(zn[:, :mw], zbs[mi][:, :mw],
                                         rn[:, :mw])
                    nc.scalar.activation(hTn[:, m0:m0 + mw], zn[:, :mw],
                                         mybir.ActivationFunctionType.Tanh)
                    if hnat is not None:
                        for jj in range(4):
                            j = mi * 4 + jj
                            pt = t_ps.tile([128, H], F32, tag="pt")
                            nc.tensor.transpose(
                                pt[:, :], hTn[:, j * 128:(j + 1) * 128],
                                i32[:, :])
                            nc.vector.tensor_copy(hnat[:, j, :], pt[:, :])
                        agr = ag_in[li].ap().rearrange(
                            "(j p) f -> p j f", p=128)
                        nc.gpsimd.dma_start(
                            agr[:, mi * 4:(mi + 1) * 4, :],
                            hnat[:, mi * 4:(mi + 1) * 4, :])
                if li == 2:
                    return hTn, None
                nc.gpsimd.collective_compute(
                    "AllGather", mybir.AluOpType.bypass, replica_groups=rg,
                    ins=[ag_in[li].ap().opt()], outs=[ag_out[li].ap().opt()])
                # keep the PE HAM-warm through the collective stall: a chain
                # of dependency-free matmuls on resident x data into a spare
                # psum slot (otherwise the PE re-throttles to 1.2 GHz and the
                # next layer runs cold and PE-bound).
                pw = bc_ps.tile([H, 512], F32, tag="pbc", name=f"warm{li}")
                for dmy in range(48):
                    nc.tensor.matmul(pw[:, :], xs[:, 0, 0:H], xs[:, 0:4, :],
                                     start=(dmy == 0), stop=(dmy == 47))
                h_stat_n = hstatp.tile([128, KC, H], DT_STAT, tag="hstat",
                                        name=f"hstat{li}")
                agor = ag_out[li].ap().rearrange("(k p) f -> p k f", p=128)
                for g in range(0, KC, 8):
                    nc.gpsimd.dma_start(h_stat_n[:, g:g + 8, :],
                                        agor[:, g:g + 8, :])
                return hTn, h_stat_n

            hT1, hs1 = layer(0, F, None, xt, w1t, w1b, b1)
            hT2, hs2 = layer(1, H, hs1, hT1, w2t, w2b, b2)
            hT3, _ = layer(2, H, hs2, hT2, w3t, w3b, b3)

            # global sum pool over this shard's nodes (padded nodes are 0)
            pT = ep.tile([H, 1], F32, tag="pT")
            nc.vector.reduce_sum(pT[:, :], hT3[:, :], axis=mybir.AxisListType.X)
            nc.gpsimd.dma_start(ar_in[:, :], pT[:])
            nc.gpsimd.collective_compute(
                "AllReduce", mybir.AluOpType.add, replica_groups=rg,
                ins=[ar_in.ap().opt()], outs=[ar_out.ap().opt()])
            pS = ep.tile([H, 1], F32, tag="pS")
            nc.gpsimd.dma_start(pS[:], ar_out[:, :])

            # final MLP (redundant on every core)
            pq = z_ps.tile([2 * H, 1], F32, tag="pz")
            nc.tensor.matmul(pq[:, :], wf1[:, :], pS[:, :], start=True, stop=True)
            q = ep.tile([2 * H, 1], F32, tag="q")
            nc.scalar.activation(q[:, :], pq[:, :],
                                 mybir.ActivationFunctionType.Tanh,
                                 bias=bf1[:])
            po = z_ps.tile([1, 1], F32, tag="pz")
            nc.tensor.matmul(po[:, :], wf2[:, :], q[:, :], start=True, stop=True)
            ob = ep.tile([1, 1], F32, tag="ob")
            nc.vector.tensor_scalar_add(ob[:, :], po[:, :], bf2[:])
            nc.gpsimd.dma_start(out_d[:, :], ob[:])

    nc.compile()
    return nc


# ---------------------------------------------------------------- host prep
def _prep(inputs):
    x = np.asarray(inputs["x"], np.float32)
    a = np.asarray(inputs["a"], np.float32)
    diag = np.diagonal(a).copy()
    add = (np.abs(diag) < TOL).astype(np.float32)
    deg = a.sum(axis=1) + add          # row sums of a_hat
    recip = np.ones(NP, np.float32)
    recip[:N] = 1.0 / deg

    x_pad = np.zeros((NP, F), np.float32)
    x_pad[:N] = x
    xs = x_pad.astype(NP_STAT)

    w1 = np.asarray(inputs["W1"], np.float32)
    common = {
        "xs": xs,
        "w1t": w1[:F].copy(), "w1b": w1[F:].copy(),
        "w2t": np.asarray(inputs["W2"], np.float32)[:H].copy(),
        "w2b": np.asarray(inputs["W2"], np.float32)[H:].copy(),
        "w3t": np.asarray(inputs["W3"], np.float32)[:H].copy(),
        "w3b": np.asarray(inputs["W3"], np.float32)[H:].copy(),
        "wf1": np.asarray(inputs["Wf1"], np.float32),
        "wf2": np.asarray(inputs["Wf2"], np.float32),
        "b1": np.asarray(inputs["b1"], np.float32).reshape(H, 1),
        "b2": np.asarray(inputs["b2"], np.float32).reshape(H, 1),
        "b3": np.asarray(inputs["b3"], np.float32).reshape(H, 1),
        "bf1": np.asarray(inputs["bf1"], np.float32).reshape(2 * H, 1),
        "bf2": np.asarray(inputs["bf2"], np.float32).reshape(1, 1),
        "i32": np.eye(32, dtype=np.float32),
    }

    in_maps = []
    for c in range(NC):
        r0 = c * SH
        r1 = min((c + 1) * SH, N)
        nrow = max(r1 - r0, 0)
        at = np.zeros((NP, SH), NP_BIG)
        if nrow > 0:
            blk = a[r0:r1].T.astype(NP_BIG)         # [N(12000), nrow]
            at[:N, :nrow] = blk
            # self-loops on approximately-zero diagonal entries
            idx = np.arange(nrow)
            gi = r0 + idx
            sel = add[gi] > 0
            at[gi[sel], idx[sel]] = np.asarray(
                a[gi[sel], gi[sel]] + 1.0, NP_BIG)
        xt = np.zeros((F, SH), np.float32)
        if nrow > 0:
            xt[:, :nrow] = x[r0:r1].T
        rcb = np.broadcast_to(recip[r0:r0 + SH], (F, SH)).copy()
        m = dict(common)
        m.update({"at": at, "xt": xt, "rc": rcb})
        in_maps.append(m)
    return in_maps


# -------------------------------------------------------------------- kernel
def kernel(**inputs):
    global LAST_EXEC_NS
    if "nc" not in _CACHE:
        _CACHE["nc"] = _build()
    nc = _CACHE["nc"]
    in_maps = _prep(inputs)
    res = run_bass_kernel_spmd(nc, in_maps, core_ids=list(range(NC)))
    LAST_EXEC_NS = res.exec_time_ns
    return np.asarray(res.results[0]["out"], np.float32).reshape(1, 1)



# revision 6
# speedup vs baseline: 2.0149x; 2.0149x over previous
"""GraphSage 3-layer GNN on 8 Trainium2 NeuronCores.

Strategy: shard nodes (rows of A) across the 8 cores. The binary 0/1
adjacency is exact in fp8e4, so the A^T shard (18 MB) is streamed from
DRAM once, pair-packed for DoubleRow, and stays RESIDENT in SBUF for
all three layers -- layers 2/3 do zero HBM traffic for A. All
aggregation matmuls run in fp8 DoubleRow mode (two 128-row k-chunks
per pass). Dense layers + l2norm(Rsqrt) + tanh run in the transposed
[feat, node] bf16 layout; AllGathers exchange fp8 h between layers in
partition-major layout (contiguous DMA lines both directions) and an
AllReduce combines the global-sum-pool partials.
"""

import os
import sys
import types

import numpy as np

# ---------------------------------------------------------------- ntff hook
# The image lacks antenv.axon_hooks; inject it so trace=True (profiling,
# enabled via BASS_TRACE=1 by test.py) can capture NTFF under axon.
def _install_ntff_hook():
    if "antenv.axon_hooks" in sys.modules:
        return
    try:
        import antenv
        mod = types.ModuleType("antenv.axon_hooks")
        _hook = [None]
        mod.set_axon_ntff_profile_hook = lambda h: _hook.__setitem__(0, h)
        mod.get_axon_ntff_profile_hook = lambda: _hook[0]
        sys.modules["antenv.axon_hooks"] = mod
        antenv.axon_hooks = mod
        from trn_agent_boot.trn_boot import _ntff_profile_via_ctypes
        so = "/opt/axon/libaxon_pjrt.so"
        if os.path.exists(so):
            mod.set_axon_ntff_profile_hook(_ntff_profile_via_ctypes(so))
    except Exception:
        pass


_install_ntff_hook()

import ml_dtypes  # noqa: E402
import concourse.bass as bass  # noqa: E402
import concourse.bacc as bacc  # noqa: E402
import concourse.tile as tile  # noqa: E402
import concourse.mybir as mybir  # noqa: E402
from concourse.bass_utils import run_bass_kernel_spmd  # noqa: E402

# ------------------------------------------------------------------ geometry
N = 12000          # real nodes
F = 128            # input feature dim
H = 32             # hidden dim
NC = 8             # cores
NP = 12288         # padded nodes  (= 96*128 = 8*1536)
SH = NP // NC      # 1536 rows per core
KC = 94            # contraction chunks with any real data (94*128 = 12032)
KP = KC // 2       # 47 DoubleRow chunk pairs
NR = KC * 128      # 12032 rows of A^T actually stored
MT = [(0, 512), (512, 512), (1024, 512)]   # m-tiles within the shard
NJ = SH // 128     # 12 transpose subtiles
KG = NP // 128 // NC   # 12 chunks contributed per core to the gather
TOL = 1e-6
WARM = int(os.environ.get("KWARM", "48"))

F32 = mybir.dt.float32
BF16 = mybir.dt.bfloat16
FP8 = mybir.dt.float8e4
NP_FP8 = ml_dtypes.float8_e4m3fn
NP_BF16 = ml_dtypes.bfloat16
DR = mybir.MatmulPerfMode.DoubleRow

LAST_EXEC_NS = None
_CACHE = {}


# ------------------------------------------------------------------- builder
def _build():
    nc = bacc.Bacc("TRN2", target_bir_lowering=False, debug=False,
                   num_devices=NC)

    # pair-packed A^T shard: [128, j(47), t(2), s(1536)] so each DMA line is
    # 3072 B contiguous and each [128, 2, 512] slice is a DoubleRow moving AP
    at_d = nc.dram_tensor("at", [128, KP * 2 * SH], FP8, kind="ExternalInput")
    xs_d = nc.dram_tensor("xs", [128, KC * F], FP8, kind="ExternalInput")
    xt_d = nc.dram_tensor("xt", [F, SH], BF16, kind="ExternalInput")
    rr_d = nc.dram_tensor("rr", [1, SH], F32, kind="ExternalInput")
    w1t_d = nc.dram_tensor("w1t", [F, H], BF16, kind="ExternalInput")
    w1b_d = nc.dram_tensor("w1b", [F, H], BF16, kind="ExternalInput")
    w2t_d = nc.dram_tensor("w2t", [H, H], BF16, kind="ExternalInput")
    w2b_d = nc.dram_tensor("w2b", [H, H], BF16, kind="ExternalInput")
    w3t_d = nc.dram_tensor("w3t", [H, H], BF16, kind="ExternalInput")
    w3b_d = nc.dram_tensor("w3b", [H, H], BF16, kind="ExternalInput")
    wf1_d = nc.dram_tensor("wf1", [H, 2 * H], F32, kind="ExternalInput")
    wf2_d = nc.dram_tensor("wf2", [2 * H, 1], F32, kind="ExternalInput")
    b1_d = nc.dram_tensor("b1", [H, 1], F32, kind="ExternalInput")
    b2_d = nc.dram_tensor("b2", [H, 1], F32, kind="ExternalInput")
    b3_d = nc.dram_tensor("b3", [H, 1], F32, kind="ExternalInput")
    bf1_d = nc.dram_tensor("bf1", [2 * H, 1], F32, kind="ExternalInput")
    bf2_d = nc.dram_tensor("bf2", [1, 1], F32, kind="ExternalInput")
    i32_d = nc.dram_tensor("i32", [32, 32], BF16, kind="ExternalInput")
    out_d = nc.dram_tensor("out", [1, 1], F32, kind="ExternalOutput")

    # partition-major gather buffers: per-core block is [128, KG*H] with
    # contiguous per-partition lines on both the send and receive side
    ag_in = [nc.dram_tensor(f"ag_in{l}", [128, KG * H], FP8) for l in range(2)]
    ag_out = [nc.dram_tensor(f"ag_out{l}", [NC * 128, KG * H], FP8,
                             addr_space="Shared") for l in range(2)]
    ar_in = nc.dram_tensor("ar_in", [H, 1], F32)
    ar_out = nc.dram_tensor("ar_out", [H, 1], F32, addr_space="Shared")
    rg = [list(range(NC))]

    with tile.TileContext(nc) as tc:
        with (
            tc.tile_pool(name="const", bufs=1) as constp,
            tc.tile_pool(name="abig", bufs=1) as abigp,
            tc.tile_pool(name="xstat", bufs=1) as xstatp,
            tc.tile_pool(name="hstat", bufs=1) as hstatp,
            tc.tile_pool(name="hT", bufs=2) as hTp,
            tc.tile_pool(name="hnat", bufs=1) as hnatp,
            tc.tile_pool(name="rcb", bufs=1) as rcbp,
            tc.tile_pool(name="p3", bufs=3) as p3,
            tc.tile_pool(name="p2", bufs=2) as p2,
            tc.tile_pool(name="agg_ps", bufs=3, space=bass.MemorySpace.PSUM) as agg_ps,
            tc.tile_pool(name="z_ps", bufs=2, space=bass.MemorySpace.PSUM) as z_ps,
            tc.tile_pool(name="bc_ps", bufs=1, space=bass.MemorySpace.PSUM) as bc_ps,
            tc.tile_pool(name="t_ps", bufs=2, space=bass.MemorySpace.PSUM) as t_ps,
        ):
            # first x-stationary group before everything else on gpsimd
            xs = xstatp.tile([128, KC, F], FP8)
            xs_r = xs_d.ap().rearrange("p (k f) -> p k f", f=F)
            nc.gpsimd.dma_start(xs[:, 0:8, :], xs_r[:, 0:8, :])

            # resident pair-packed A^T; stream the whole shard on sync queue
            at4 = abigp.tile([128, KP, 2, SH], FP8)
            at_r = at_d.ap().rearrange("p (j t s) -> p j t s", t=2, s=SH)
            for j in range(KP):
                nc.sync.dma_start(at4[:, j, :, :], at_r[:, j, :, :])

            # ---- constants
            def cload(dram, shape, dt=F32):
                t = constp.tile(shape, dt, tag=dram.name)
                nc.gpsimd.dma_start(t[:], dram[:, :])
                return t

            w1t = cload(w1t_d, [F, H], BF16)
            w1b = cload(w1b_d, [F, H], BF16)
            w2t = cload(w2t_d, [H, H], BF16)
            w2b = cload(w2b_d, [H, H], BF16)
            w3t = cload(w3t_d, [H, H], BF16)
            w3b = cload(w3b_d, [H, H], BF16)
            wf1 = cload(wf1_d, [H, 2 * H])
            wf2 = cload(wf2_d, [2 * H, 1])
            b1 = cload(b1_d, [H, 1])
            b2 = cload(b2_d, [H, 1])
            b3 = cload(b3_d, [H, 1])
            bf1 = cload(bf1_d, [2 * H, 1])
            bf2 = cload(bf2_d, [1, 1])
            i32 = cload(i32_d, [32, 32], BF16)
            xt = cload(xt_d, [F, SH], BF16)
            rrow = cload(rr_d, [1, SH])

            ones_m = constp.tile([H, H], BF16, tag="ones_m")
            nc.gpsimd.memset(ones_m[:], 1.0)
            ones1 = constp.tile([1, 128], F32, tag="ones1")
            nc.gpsimd.memset(ones1[:], 1.0)

            # remaining x-stationary groups
            for g in range(8, KC, 8):
                ge = min(g + 8, KC)
                nc.gpsimd.dma_start(xs[:, g:ge, :], xs_r[:, g:ge, :])

            # broadcast 1/deg row to all 128 partitions via ones-matmul
            rc = rcbp.tile([F, SH], F32)
            for m0, mw in MT:
                prc = bc_ps.tile([F, 512], F32, tag="pbc", name=f"prc{m0}")
                nc.tensor.matmul(prc[:, :mw], ones1[:, :],
                                 rrow[:, m0:m0 + mw], start=True, stop=True)
                nc.vector.tensor_copy(rc[:, m0:m0 + mw], prc[:, :mw])

            def layer(li, fl, h_stat, hT, wtop, wbot, b):
                """one SageConv layer; returns hT_next [H, SH] bf16 tile."""
                hTn = hTp.tile([H, SH], BF16, tag="hTn")
                # big aggregation matmul in fp8 DoubleRow: pair-outer, three
                # 512-col matmuls accumulate into three psum banks. A is
                # resident in SBUF (streamed from HBM during layer 0 only).
                paggs = []
                for _mi in range(len(MT)):
                    paggs.append(agg_ps.tile([fl, 512], F32, tag="pagg",
                                             name=f"pagg{li}_{_mi}"))
                for j in range(KP):
                    hj = h_stat[:, 2 * j:2 * j + 2, :]
                    for mi, (m0, mw) in enumerate(MT):
                        nc.tensor.matmul(
                            paggs[mi][:, :mw], hj, at4[:, j, :, m0:m0 + mw],
                            start=(j == 0), stop=(j == KP - 1), perf_mode=DR)
                zbs, sss = [], []
                for mi, (m0, mw) in enumerate(MT):
                    pagg = paggs[mi]
                    # scaled aggregation (mean): agg^T * (1/deg) broadcast
                    aggs = p2.tile([F, 512], BF16, tag="aggs")
                    nc.vector.tensor_mul(
                        aggs[:fl, :mw], pagg[:fl, :mw], rc[:fl, m0:m0 + mw])
                    # z^T = Wtop^T h^T + Wbot^T agg^T
                    pz = z_ps.tile([H, 512], F32, tag="pz")
                    nc.tensor.matmul(pz[:, :mw], wtop[:, :], hT[:, m0:m0 + mw],
                                     start=True, stop=False)
                    nc.tensor.matmul(pz[:, :mw], wbot[:, :], aggs[:fl, :mw],
                                     start=False, stop=True)
                    zb = p3.tile([H, 512], BF16, tag="zb")
                    nc.vector.tensor_scalar_add(zb[:, :mw], pz[:, :mw], b[:])
                    # row l2-norm over features (partition dim): sumsq via
                    # ones-matmul, broadcast back to H partitions, then one
                    # fused Rsqrt at [H, mw] width (one activation-table load
                    # per layer instead of per tile).
                    sq = p2.tile([H, 512], BF16, tag="sq")
                    nc.vector.tensor_mul(sq[:, :mw], zb[:, :mw], zb[:, :mw])
                    pbc = bc_ps.tile([H, 512], F32, tag="pbc")
                    nc.tensor.matmul(pbc[:, :mw], ones_m[:, :], sq[:, :mw],
                                     start=True, stop=True)
                    ssb = p3.tile([H, 512], BF16, tag="ssb")
                    nc.vector.tensor_scalar_max(ssb[:, :mw], pbc[:, :mw], 1e-12)
                    zbs.append(zb)
                    sss.append(ssb)
                srts = []
                for mi, (m0, mw) in enumerate(MT):
                    srt = p3.tile([H, 512], F32, tag="srt")
                    nc.scalar.sqrt(srt[:, :mw], sss[mi][:, :mw])
                    srts.append(srt)
                rns = []
                for mi, (m0, mw) in enumerate(MT):
                    rn = p3.tile([H, 512], F32, tag="rn")
                    nc.vector.reciprocal_approx_fast(rn[:, :mw],
                                                     srts[mi][:, :mw])
                    rns.append(rn)
                if li < 2:
                    hnat = hnatp.tile([128, NJ, H], FP8, tag="hnat",
                                      name=f"hnat{li}")
                else:
                    hnat = None
                for mi, (m0, mw) in enumerate(MT):
                    zn = p2.tile([H, 512], BF16, tag="zn")
                    nc.vector.tensor_mul(zn[:, :mw], zbs[mi][:, :mw],
                                         rns[mi][:, :mw])
                    nc.scalar.activation(hTn[:, m0:m0 + mw], zn[:, :mw],
                                         mybir.ActivationFunctionType.Tanh)
                    if hnat is not None:
                        for jj in range(4):
                            j = mi * 4 + jj
                            pt = t_ps.tile([128, H], BF16, tag="pt")
                            nc.tensor.transpose(
                                pt[:, :], hTn[:, j * 128:(j + 1) * 128],
                                i32[:, :])
                            nc.vector.tensor_copy(hnat[:, j, :], pt[:, :])
                        agr = ag_in[li].ap().rearrange("p (j f) -> p j f", f=H)
                        nc.gpsimd.dma_start(
                            agr[:, mi * 4:(mi + 1) * 4, :],
                            hnat[:, mi * 4:(mi + 1) * 4, :])
                if li == 2:
                    return hTn, None
                nc.gpsimd.collective_compute(
                    "AllGather", mybir.AluOpType.bypass, replica_groups=rg,
                    ins=[ag_in[li].ap().opt()], outs=[ag_out[li].ap().opt()])
                # keep the PE HAM-warm through the collective stall: a chain
                # of dependency-free matmuls on resident A data into a spare
                # psum slot (otherwise the PE re-throttles to 1.2 GHz and the
                # next layer starts cold and PE-bound).
                pw = bc_ps.tile([H, 512], F32, tag="pbc", name=f"warm{li}")
                for dmy in range(WARM):
                    nc.tensor.matmul(pw[:, :], at4[:, 0, 0, 0:H],
                                     at4[:, 0, 1, 0:512],
                                     start=(dmy == 0), stop=(dmy == WARM - 1))
                # gathered h in partition-major per-core blocks: one
                # contiguous DMA per peer core (384 B lines)
                h_stat_n = hstatp.tile([128, NC * KG, H], FP8, tag="hstat",
                                       name=f"hstat{li}")
                agor = ag_out[li].ap().rearrange("(c p) (j f) -> c p j f",
                                                 p=128, f=H)
                for c in range(NC):
                    nc.gpsimd.dma_start(h_stat_n[:, c * KG:(c + 1) * KG, :],
                                        agor[c, :, :, :])
                return hTn, h_stat_n

            hT1, hs1 = layer(0, F, xs, xt, w1t, w1b, b1)
            hT2, hs2 = layer(1, H, hs1, hT1, w2t, w2b, b2)
            hT3, _ = layer(2, H, hs2, hT2, w3t, w3b, b3)

            # global sum pool over this shard's nodes (padded nodes are 0)
            pT = p2.tile([H, 1], F32, tag="pT")
            nc.vector.reduce_sum(pT[:, :], hT3[:, :], axis=mybir.AxisListType.X)
            nc.gpsimd.dma_start(ar_in[:, :], pT[:])
            nc.gpsimd.collective_compute(
                "AllReduce", mybir.AluOpType.add, replica_groups=rg,
                ins=[ar_in.ap().opt()], outs=[ar_out.ap().opt()])
            pS = p2.tile([H, 1], F32, tag="pS")
            nc.gpsimd.dma_start(pS[:], ar_out[:, :])

            # final MLP (redundant on every core)
            pq = z_ps.tile([2 * H, 1], F32, tag="pz")
            nc.tensor.matmul(pq[:, :], wf1[:, :], pS[:, :], start=True, stop=True)
            q = p2.tile([2 * H, 1], F32, tag="q")
            nc.scalar.activation(q[:, :], pq[:, :],
                                 mybir.ActivationFunctionType.Tanh,
                                 bias=bf1[:])
            po = z_ps.tile([1, 1], F32, tag="pz")
            nc.tensor.matmul(po[:, :], wf2[:, :], q[:, :], start=True, stop=True)
            ob = p2.tile([1, 1], F32, tag="ob")
            nc.vector.tensor_scalar_add(ob[:, :], po[:, :], bf2[:])
            nc.gpsimd.dma_start(out_d[:, :], ob[:])

    nc.compile()
    return nc


# ---------------------------------------------------------------- host prep
def _prep(inputs):
    x = np.asarray(inputs["x"], np.float32)
    a = np.asarray(inputs["a"], np.float32)
    diag = np.diagonal(a).copy()
    add = (np.abs(diag) < TOL).astype(np.float32)
    deg = a.sum(axis=1) + add          # row sums of a_hat
    recip = np.ones(SH, np.float32)

    x_pad = np.zeros((NR, F), np.float32)
    x_pad[:N] = x
    # partition-major x chunks: xs[p, k, f] = x[k*128 + p, f]
    xs = np.ascontiguousarray(
        x_pad.reshape(KC, 128, F).transpose(1, 0, 2)).reshape(128, KC * F)
    xs = xs.astype(NP_FP8)

    w1 = np.asarray(inputs["W1"], np.float32)
    common = {
        "xs": xs,
        "w1t": w1[:F].astype(NP_BF16), "w1b": w1[F:].astype(NP_BF16),
        "w2t": np.asarray(inputs["W2"], np.float32)[:H].astype(NP_BF16),
        "w2b": np.asarray(inputs["W2"], np.float32)[H:].astype(NP_BF16),
        "w3t": np.asarray(inputs["W3"], np.float32)[:H].astype(NP_BF16),
        "w3b": np.asarray(inputs["W3"], np.float32)[H:].astype(NP_BF16),
        "wf1": np.asarray(inputs["Wf1"], np.float32),
        "wf2": np.asarray(inputs["Wf2"], np.float32),
        "b1": np.asarray(inputs["b1"], np.float32).reshape(H, 1),
        "b2": np.asarray(inputs["b2"], np.float32).reshape(H, 1),
        "b3": np.asarray(inputs["b3"], np.float32).reshape(H, 1),
        "bf1": np.asarray(inputs["bf1"], np.float32).reshape(2 * H, 1),
        "bf2": np.asarray(inputs["bf2"], np.float32).reshape(1, 1),
        "i32": np.eye(32, dtype=NP_BF16),
    }

    in_maps = []
    for c in range(NC):
        r0 = c * SH
        r1 = min((c + 1) * SH, N)
        nrow = max(r1 - r0, 0)
        at = np.zeros((NR, SH), np.float32)
        if nrow > 0:
            at[:N, :nrow] = a[r0:r1].T          # [N(12000), nrow]
            # self-loops on approximately-zero diagonal entries
            idx = np.arange(nrow)
            gi = r0 + idx
            sel = add[gi] > 0
            at[gi[sel], idx[sel]] = a[gi[sel], gi[sel]] + 1.0
        # pair-packed fp8: at4[p, j, t, s] = at[(2j+t)*128 + p, s]
        at4 = np.ascontiguousarray(
            at.reshape(KP, 2, 128, SH).transpose(2, 0, 1, 3)
        ).reshape(128, KP * 2 * SH).astype(NP_FP8)
        xt = np.zeros((F, SH), NP_BF16)
        if nrow > 0:
            xt[:, :nrow] = x[r0:r1].T.astype(NP_BF16)
        rr = recip.copy()
        if nrow > 0:
            rr[:nrow] = 1.0 / deg[r0:r1]
        m = dict(common)
        m.update({"at": at4, "xt": xt, "rr": rr.reshape(1, SH)})
        in_maps.append(m)
    return in_maps


# -------------------------------------------------------------------- kernel
def kernel(**inputs):
    global LAST_EXEC_NS
    if "nc" not in _CACHE:
        _CACHE["nc"] = _build()
    nc = _CACHE["nc"]
    in_maps = _prep(inputs)
    res = run_bass_kernel_spmd(nc, in_maps, core_ids=list(range(NC)))
    LAST_EXEC_NS = res.exec_time_ns
    return np.asarray(res.results[0]["out"], np.float32).reshape(1, 1)


# revision 9
# speedup vs baseline: 2.1416x; 1.0629x over previous
"""GraphSage 3-layer GNN on 8 Trainium2 NeuronCores.

Strategy: shard nodes (rows of A) across the 8 cores. The binary 0/1
adjacency is exact in fp8e4, so the A^T shard (18 MB) is streamed from
DRAM once, pair-packed for DoubleRow, and stays RESIDENT in SBUF for
all three layers -- layers 2/3 do zero HBM traffic for A. All
aggregation matmuls run in fp8 DoubleRow mode (two 128-row k-chunks
per pass). Dense layers + l2norm(Rsqrt) + tanh run in the transposed
[feat, node] bf16 layout; AllGathers exchange fp8 h between layers in
partition-major layout (contiguous DMA lines both directions) and an
AllReduce combines the global-sum-pool partials.
"""

import os
import sys
import types

import numpy as np

# ---------------------------------------------------------------- ntff hook
# The image lacks antenv.axon_hooks; inject it so trace=True (profiling,
# enabled via BASS_TRACE=1 by test.py) can capture NTFF under axon.
def _install_ntff_hook():
    if "antenv.axon_hooks" in sys.modules:
        return
    try:
        import antenv
        mod = types.ModuleType("antenv.axon_hooks")
        _hook = [None]
        mod.set_axon_ntff_profile_hook = lambda h: _hook.__setitem__(0, h)
        mod.get_axon_ntff_profile_hook = lambda: _hook[0]
        sys.modules["antenv.axon_hooks"] = mod
        antenv.axon_hooks = mod
        from trn_agent_boot.trn_boot import _ntff_profile_via_ctypes
        so = "/opt/axon/libaxon_pjrt.so"
        if os.path.exists(so):
            mod.set_axon_ntff_profile_hook(_ntff_profile_via_ctypes(so))
    except Exception:
        pass


_install_ntff_hook()

import ml_dtypes  # noqa: E402
import concourse.bass as bass  # noqa: E402
import concourse.bacc as bacc  # noqa: E402
import concourse.tile as tile  # noqa: E402
import concourse.mybir as mybir  # noqa: E402
from concourse.bass_utils import run_bass_kernel_spmd  # noqa: E402

# ------------------------------------------------------------------ geometry
N = 12000          # real nodes
F = 128            # input feature dim
H = 32             # hidden dim
NC = 8             # cores
NP = 12288         # padded nodes  (= 96*128 = 8*1536)
SH = NP // NC      # 1536 rows per core
KC = 94            # contraction chunks with any real data (94*128 = 12032)
KP = KC // 2       # 47 DoubleRow chunk pairs
NR = KC * 128      # 12032 rows of A^T actually stored
MT = [(0, 512), (512, 512), (1024, 512)]   # m-tiles within the shard
NJ = SH // 128     # 12 transpose subtiles
KG = NP // 128 // NC   # 12 chunks contributed per core to the gather
TOL = 1e-6
WARM = int(os.environ.get("KWARM", "0"))

F32 = mybir.dt.float32
BF16 = mybir.dt.bfloat16
FP8 = mybir.dt.float8e4
NP_FP8 = ml_dtypes.float8_e4m3fn
NP_BF16 = ml_dtypes.bfloat16
DR = mybir.MatmulPerfMode.DoubleRow

LAST_EXEC_NS = None
_CACHE = {}


# ------------------------------------------------------------------- builder
def _build():
    nc = bacc.Bacc("TRN2", target_bir_lowering=False, debug=False,
                   num_devices=NC)

    # pair-packed A^T shard: [128, j(47), t(2), s(1536)] so each DMA line is
    # 3072 B contiguous and each [128, 2, 512] slice is a DoubleRow moving AP
    at_d = nc.dram_tensor("at", [128, KP * 2 * SH], FP8, kind="ExternalInput")
    xs_d = nc.dram_tensor("xs", [128, KC * F], FP8, kind="ExternalInput")
    xt_d = nc.dram_tensor("xt", [F, SH], BF16, kind="ExternalInput")
    rr_d = nc.dram_tensor("rr", [1, SH], F32, kind="ExternalInput")
    w1t_d = nc.dram_tensor("w1t", [F, H], BF16, kind="ExternalInput")
    w1b_d = nc.dram_tensor("w1b", [F, H], BF16, kind="ExternalInput")
    w2t_d = nc.dram_tensor("w2t", [H, H], BF16, kind="ExternalInput")
    w2b_d = nc.dram_tensor("w2b", [H, H], BF16, kind="ExternalInput")
    w3t_d = nc.dram_tensor("w3t", [H, H], BF16, kind="ExternalInput")
    w3b_d = nc.dram_tensor("w3b", [H, H], BF16, kind="ExternalInput")
    wf1_d = nc.dram_tensor("wf1", [H, 2 * H], F32, kind="ExternalInput")
    wf2_d = nc.dram_tensor("wf2", [2 * H, 1], F32, kind="ExternalInput")
    b1_d = nc.dram_tensor("b1", [1, H], BF16, kind="ExternalInput")
    b2_d = nc.dram_tensor("b2", [1, H], BF16, kind="ExternalInput")
    b3_d = nc.dram_tensor("b3", [1, H], BF16, kind="ExternalInput")
    bf1_d = nc.dram_tensor("bf1", [2 * H, 1], F32, kind="ExternalInput")
    bf2_d = nc.dram_tensor("bf2", [1, 1], F32, kind="ExternalInput")
    i32_d = nc.dram_tensor("i32", [32, 32], BF16, kind="ExternalInput")
    out_d = nc.dram_tensor("out", [1, 1], F32, kind="ExternalOutput")

    # partition-major gather buffers: per-core block is [128, KG*H] with
    # contiguous per-partition lines on both the send and receive side
    ag_in = [nc.dram_tensor(f"ag_in{l}", [128, KG * H], FP8) for l in range(2)]
    ag_out = [nc.dram_tensor(f"ag_out{l}", [NC * 128, KG * H], FP8,
                             addr_space="Shared") for l in range(2)]
    ar_in = nc.dram_tensor("ar_in", [H, 1], F32)
    ar_out = nc.dram_tensor("ar_out", [H, 1], F32, addr_space="Shared")
    rg = [list(range(NC))]

    with tile.TileContext(nc) as tc:
        with (
            tc.tile_pool(name="const", bufs=1) as constp,
            tc.tile_pool(name="abig", bufs=1) as abigp,
            tc.tile_pool(name="xstat", bufs=1) as xstatp,
            tc.tile_pool(name="hstat", bufs=1) as hstatp,
            tc.tile_pool(name="hT", bufs=2) as hTp,
            tc.tile_pool(name="hnat", bufs=1) as hnatp,
            tc.tile_pool(name="rcb", bufs=1) as rcbp,
            tc.tile_pool(name="p3", bufs=3) as p3,
            tc.tile_pool(name="p2", bufs=2) as p2,
            tc.tile_pool(name="agg_ps", bufs=3, space=bass.MemorySpace.PSUM) as agg_ps,
            tc.tile_pool(name="z_ps", bufs=3, space=bass.MemorySpace.PSUM) as z_ps,
            tc.tile_pool(name="t_ps", bufs=2, space=bass.MemorySpace.PSUM) as t_ps,
        ):
            # first x-stationary group before everything else on gpsimd
            xs = xstatp.tile([128, KC, F], FP8)
            xs_r = xs_d.ap().rearrange("p (k f) -> p k f", f=F)
            nc.gpsimd.dma_start(xs[:, 0:8, :], xs_r[:, 0:8, :])

            # resident pair-packed A^T; stream the whole shard on sync queue
            at4 = abigp.tile([128, KP, 2, SH], FP8)
            at_r = at_d.ap().rearrange("p (j t s) -> p j t s", t=2, s=SH)
            for j in range(KP):
                nc.sync.dma_start(at4[:, j, :, :], at_r[:, j, :, :])

            # ---- constants
            def cload(dram, shape, dt=F32):
                t = constp.tile(shape, dt, tag=dram.name)
                nc.gpsimd.dma_start(t[:], dram[:, :])
                return t

            w1t = cload(w1t_d, [F, H], BF16)
            w1b = cload(w1b_d, [F, H], BF16)
            w2t = cload(w2t_d, [H, H], BF16)
            w2b = cload(w2b_d, [H, H], BF16)
            w3t = cload(w3t_d, [H, H], BF16)
            w3b = cload(w3b_d, [H, H], BF16)
            wf1 = cload(wf1_d, [H, 2 * H])
            wf2 = cload(wf2_d, [2 * H, 1])
            b1 = cload(b1_d, [1, H], BF16)
            b2 = cload(b2_d, [1, H], BF16)
            b3 = cload(b3_d, [1, H], BF16)
            bf1 = cload(bf1_d, [2 * H, 1])
            bf2 = cload(bf2_d, [1, 1])
            i32 = cload(i32_d, [32, 32], BF16)
            xt = cload(xt_d, [F, SH], BF16)
            rrow = cload(rr_d, [1, SH])

            ones_m = constp.tile([H, H], BF16, tag="ones_m")
            nc.gpsimd.memset(ones_m[:], 1.0)
            ones1 = constp.tile([1, 128], F32, tag="ones1")
            nc.gpsimd.memset(ones1[:], 1.0)
            onesb = constp.tile([1, 512], BF16, tag="onesb")
            nc.gpsimd.memset(onesb[:], 1.0)
            epsr = constp.tile([1, H], BF16, tag="epsr")
            nc.gpsimd.memset(epsr[:], 1e-12)

            # remaining x-stationary groups
            for g in range(8, KC, 8):
                ge = min(g + 8, KC)
                nc.gpsimd.dma_start(xs[:, g:ge, :], xs_r[:, g:ge, :])

            # broadcast 1/deg row to all 128 partitions via ones-matmul
            rc = rcbp.tile([F, SH], F32)
            for m0, mw in MT:
                prc = agg_ps.tile([F, 512], F32, tag="pagg", name=f"prc{m0}")
                nc.tensor.matmul(prc[:, :mw], ones1[:, :],
                                 rrow[:, m0:m0 + mw], start=True, stop=True)
                nc.vector.tensor_copy(rc[:, m0:m0 + mw], prc[:, :mw])

            def layer(li, fl, h_stat, hT, wtop, wbot, b):
                """one SageConv layer; returns hT_next [H, SH] bf16 tile."""
                hTn = hTp.tile([H, SH], BF16, tag="hTn")
                # big aggregation matmul in fp8 DoubleRow: pair-outer, three
                # 512-col matmuls accumulate into three psum banks. A is
                # resident in SBUF (streamed from HBM during layer 0 only).
                paggs = []
                for _mi in range(len(MT)):
                    paggs.append(agg_ps.tile([fl, 512], F32, tag="pagg",
                                             name=f"pagg{li}_{_mi}"))
                for j in range(KP):
                    hj = h_stat[:, 2 * j:2 * j + 2, :]
                    for mi, (m0, mw) in enumerate(MT):
                        nc.tensor.matmul(
                            paggs[mi][:, :mw], hj, at4[:, j, :, m0:m0 + mw],
                            start=(j == 0), stop=(j == KP - 1), perf_mode=DR)
                pzs, pbcs = [], []
                for mi, (m0, mw) in enumerate(MT):
                    pagg = paggs[mi]
                    # scaled aggregation (mean): agg^T * (1/deg) broadcast
                    aggs = p2.tile([F, 512], BF16, tag="aggs")
                    nc.vector.tensor_mul(
                        aggs[:fl, :mw], pagg[:fl, :mw], rc[:fl, m0:m0 + mw])
                    # z^T = Wtop^T h^T + Wbot^T agg^T + b, with the bias as a
                    # rank-1 matmul so no extra DVE pass is needed
                    pz = z_ps.tile([H, 512], F32, tag="pz")
                    nc.tensor.matmul(pz[:, :mw], wtop[:, :], hT[:, m0:m0 + mw],
                                     start=True, stop=False)
                    nc.tensor.matmul(pz[:, :mw], wbot[:, :], aggs[:fl, :mw],
                                     start=False, stop=False)
                    nc.tensor.matmul(pz[:, :mw], b[:, :], onesb[:, :mw],
                                     start=False, stop=True)
                    # row l2-norm over features (partition dim): sumsq via
                    # ones-matmul, +1e-12 folded in as a rank-1 term standing
                    # in for the reference's max(ss, 1e-12)
                    sq = p2.tile([H, 512], BF16, tag="sq")
                    nc.scalar.square(sq[:, :mw], pz[:, :mw])
                    pbc = agg_ps.tile([H, 512], F32, tag="pagg",
                                      name=f"pbc{li}_{mi}")
                    nc.tensor.matmul(pbc[:, :mw], ones_m[:, :], sq[:, :mw],
                                     start=True, stop=False)
                    nc.tensor.matmul(pbc[:, :mw], epsr[:, :], onesb[:, :mw],
                                     start=False, stop=True)
                    pzs.append(pz)
                    pbcs.append(pbc)
                srts = []
                for mi, (m0, mw) in enumerate(MT):
                    srt = p3.tile([H, 512], F32, tag="srt")
                    nc.scalar.sqrt(srt[:, :mw], pbcs[mi][:, :mw])
                    srts.append(srt)
                if li < 2:
                    hnat = hnatp.tile([128, NJ, H], FP8, tag="hnat",
                                      name=f"hnat{li}")
                else:
                    hnat = None
                for mi, (m0, mw) in enumerate(MT):
                    rn = p2.tile([H, 512], F32, tag="rn")
                    nc.vector.reciprocal_approx_fast(rn[:, :mw],
                                                     srts[mi][:, :mw])
                    zn = p2.tile([H, 512], BF16, tag="zn")
                    nc.vector.tensor_mul(zn[:, :mw], pzs[mi][:, :mw],
                                         rn[:, :mw])
                    nc.scalar.activation(hTn[:, m0:m0 + mw], zn[:, :mw],
                                         mybir.ActivationFunctionType.Tanh)
                    if hnat is not None:
                        for jj in range(4):
                            j = mi * 4 + jj
                            pt = t_ps.tile([128, H], BF16, tag="pt")
                            nc.tensor.transpose(
                                pt[:, :], hTn[:, j * 128:(j + 1) * 128],
                                i32[:, :])
                            nc.vector.tensor_copy(hnat[:, j, :], pt[:, :])
                if hnat is not None:
                    nc.gpsimd.dma_start(
                        ag_in[li].ap().rearrange(
                            "p (j f) -> p j f", f=H)[:, :, :],
                        hnat[:, :, :])
                if li == 2:
                    return hTn, None
                nc.gpsimd.collective_compute(
                    "AllGather", mybir.AluOpType.bypass, replica_groups=rg,
                    ins=[ag_in[li].ap().opt()], outs=[ag_out[li].ap().opt()])
                if WARM > 0:
                    # optional PE keep-warm chain through the collective
                    pw = z_ps.tile([H, 512], F32, tag="pz", name=f"warm{li}")
                    for dmy in range(WARM):
                        nc.tensor.matmul(pw[:, :], at4[:, 0, 0, 0:H],
                                         at4[:, 0, 1, 0:512],
                                         start=(dmy == 0),
                                         stop=(dmy == WARM - 1))
                # gathered h in partition-major per-core blocks: one
                # contiguous DMA per peer core (384 B lines)
                h_stat_n = hstatp.tile([128, NC * KG, H], FP8, tag="hstat",
                                       name=f"hstat{li}")
                agor = ag_out[li].ap().rearrange("(c p) (j f) -> c p j f",
                                                 p=128, f=H)
                for c in range(NC):
                    nc.gpsimd.dma_start(h_stat_n[:, c * KG:(c + 1) * KG, :],
                                        agor[c, :, :, :])
                return hTn, h_stat_n

            hT1, hs1 = layer(0, F, xs, xt, w1t, w1b, b1)
            hT2, hs2 = layer(1, H, hs1, hT1, w2t, w2b, b2)
            hT3, _ = layer(2, H, hs2, hT2, w3t, w3b, b3)

            # global sum pool over this shard's nodes (padded nodes are 0)
            pT = p2.tile([H, 1], F32, tag="pT")
            nc.vector.reduce_sum(pT[:, :], hT3[:, :], axis=mybir.AxisListType.X)
            nc.gpsimd.dma_start(ar_in[:, :], pT[:])
            nc.gpsimd.collective_compute(
                "AllReduce", mybir.AluOpType.add, replica_groups=rg,
                ins=[ar_in.ap().opt()], outs=[ar_out.ap().opt()])
            pS = p2.tile([H, 1], F32, tag="pS")
            nc.gpsimd.dma_start(pS[:], ar_out[:, :])

            # final MLP (redundant on every core)
            pq = z_ps.tile([2 * H, 1], F32, tag="pz")
            nc.tensor.matmul(pq[:, :], wf1[:, :], pS[:, :], start=True, stop=True)
            q = p2.tile([2 * H, 1], F32, tag="q")
            nc.scalar.activation(q[:, :], pq[:, :],
                                 mybir.ActivationFunctionType.Tanh,
                                 bias=bf1[:])
            po = z_ps.tile([1, 1], F32, tag="pz")
            nc.tensor.matmul(po[:, :], wf2[:, :], q[:, :], start=True, stop=True)
            ob = p2.tile([1, 1], F32, tag="ob")
            nc.vector.tensor_scalar_add(ob[:, :], po[:, :], bf2[:])
            nc.gpsimd.dma_start(out_d[:, :], ob[:])

    nc.compile()
    return nc


# ---------------------------------------------------------------- host prep
def _prep(inputs):
    x = np.asarray(inputs["x"], np.float32)
    a = np.asarray(inputs["a"], np.float32)
    diag = np.diagonal(a).copy()
    add = (np.abs(diag) < TOL).astype(np.float32)
    deg = a.sum(axis=1) + add          # row sums of a_hat
    recip = np.ones(SH, np.float32)

    x_pad = np.zeros((NR, F), np.float32)
    x_pad[:N] = x
    # partition-major x chunks: xs[p, k, f] = x[k*128 + p, f]
    xs = np.ascontiguousarray(
        x_pad.reshape(KC, 128, F).transpose(1, 0, 2)).reshape(128, KC * F)
    xs = xs.astype(NP_FP8)

    w1 = np.asarray(inputs["W1"], np.float32)
    common = {
        "xs": xs,
        "w1t": w1[:F].astype(NP_BF16), "w1b": w1[F:].astype(NP_BF16),
        "w2t": np.asarray(inputs["W2"], np.float32)[:H].astype(NP_BF16),
        "w2b": np.asarray(inputs["W2"], np.float32)[H:].astype(NP_BF16),
        "w3t": np.asarray(inputs["W3"], np.float32)[:H].astype(NP_BF16),
        "w3b": np.asarray(inputs["W3"], np.float32)[H:].astype(NP_BF16),
        "wf1": np.asarray(inputs["Wf1"], np.float32),
        "wf2": np.asarray(inputs["Wf2"], np.float32),
        "b1": np.asarray(inputs["b1"], np.float32).reshape(1, H).astype(NP_BF16),
        "b2": np.asarray(inputs["b2"], np.float32).reshape(1, H).astype(NP_BF16),
        "b3": np.asarray(inputs["b3"], np.float32).reshape(1, H).astype(NP_BF16),
        "bf1": np.asarray(inputs["bf1"], np.float32).reshape(2 * H, 1),
        "bf2": np.asarray(inputs["bf2"], np.float32).reshape(1, 1),
        "i32": np.eye(32, dtype=NP_BF16),
    }

    in_maps = []
    for c in range(NC):
        r0 = c * SH
        r1 = min((c + 1) * SH, N)
        nrow = max(r1 - r0, 0)
        at = np.zeros((NR, SH), np.float32)
        if nrow > 0:
            at[:N, :nrow] = a[r0:r1].T          # [N(12000), nrow]
            # self-loops on approximately-zero diagonal entries
            idx = np.arange(nrow)
            gi = r0 + idx
            sel = add[gi] > 0
            at[gi[sel], idx[sel]] = a[gi[sel], gi[sel]] + 1.0
        # pair-packed fp8: at4[p, j, t, s] = at[(2j+t)*128 + p, s]
        at4 = np.ascontiguousarray(
            at.reshape(KP, 2, 128, SH).transpose(2, 0, 1, 3)
        ).reshape(128, KP * 2 * SH).astype(NP_FP8)
        xt = np.zeros((F, SH), NP_BF16)
        if nrow > 0:
            xt[:, :nrow] = x[r0:r1].T.astype(NP_BF16)
        rr = recip.copy()
        if nrow > 0:
            rr[:nrow] = 1.0 / deg[r0:r1]
        m = dict(common)
        m.update({"at": at4, "xt": xt, "rr": rr.reshape(1, SH)})
        in_maps.append(m)
    return in_maps


# -------------------------------------------------------------------- kernel
def kernel(**inputs):
    global LAST_EXEC_NS
    if "nc" not in _CACHE:
        _CACHE["nc"] = _build()
    nc = _CACHE["nc"]
    in_maps = _prep(inputs)
    res = run_bass_kernel_spmd(nc, in_maps, core_ids=list(range(NC)))
    LAST_EXEC_NS = res.exec_time_ns
    return np.asarray(res.results[0]["out"], np.float32).reshape(1, 1)


# revision 11
# speedup vs baseline: 2.2018x; 1.0281x over previous
"""GraphSage 3-layer GNN on 8 Trainium2 NeuronCores.

Strategy: shard nodes (rows of A) across the 8 cores. The binary 0/1
adjacency is exact in fp8e4, so the A^T shard (18 MB) is streamed from
DRAM once, pair-packed for DoubleRow, and stays RESIDENT in SBUF for
all three layers -- layers 2/3 do zero HBM traffic for A. All
aggregation matmuls run in fp8 DoubleRow mode (two 128-row k-chunks
per pass). Dense layers + l2norm(Rsqrt) + tanh run in the transposed
[feat, node] bf16 layout; AllGathers exchange fp8 h between layers in
partition-major layout (contiguous DMA lines both directions) and an
AllReduce combines the global-sum-pool partials.
"""

import os
import sys
import types

import numpy as np

# ---------------------------------------------------------------- ntff hook
# The image lacks antenv.axon_hooks; inject it so trace=True (profiling,
# enabled via BASS_TRACE=1 by test.py) can capture NTFF under axon.
def _install_ntff_hook():
    if "antenv.axon_hooks" in sys.modules:
        return
    try:
        import antenv
        mod = types.ModuleType("antenv.axon_hooks")
        _hook = [None]
        mod.set_axon_ntff_profile_hook = lambda h: _hook.__setitem__(0, h)
        mod.get_axon_ntff_profile_hook = lambda: _hook[0]
        sys.modules["antenv.axon_hooks"] = mod
        antenv.axon_hooks = mod
        from trn_agent_boot.trn_boot import _ntff_profile_via_ctypes
        so = "/opt/axon/libaxon_pjrt.so"
        if os.path.exists(so):
            mod.set_axon_ntff_profile_hook(_ntff_profile_via_ctypes(so))
    except Exception:
        pass


_install_ntff_hook()

import ml_dtypes  # noqa: E402
import concourse.bass as bass  # noqa: E402
import concourse.bacc as bacc  # noqa: E402
import concourse.tile as tile  # noqa: E402
import concourse.mybir as mybir  # noqa: E402
from concourse.bass_utils import run_bass_kernel_spmd  # noqa: E402

# ------------------------------------------------------------------ geometry
N = 12000          # real nodes
F = 128            # input feature dim
H = 32             # hidden dim
NC = 8             # cores
NP = 12288         # padded nodes  (= 96*128 = 8*1536)
SH = NP // NC      # 1536 rows per core
KC = 94            # contraction chunks with any real data (94*128 = 12032)
KP = KC // 2       # 47 DoubleRow chunk pairs
NR = KC * 128      # 12032 rows of A^T actually stored
MT = [(0, 512), (512, 512), (1024, 512)]   # m-tiles within the shard
NJ = SH // 128     # 12 transpose subtiles
KG = NP // 128 // NC   # 12 chunks contributed per core to the gather
TOL = 1e-6
WARM = int(os.environ.get("KWARM", "0"))

F32 = mybir.dt.float32
BF16 = mybir.dt.bfloat16
FP8 = mybir.dt.float8e4
NP_FP8 = ml_dtypes.float8_e4m3fn
NP_BF16 = ml_dtypes.bfloat16
DR = mybir.MatmulPerfMode.DoubleRow

LAST_EXEC_NS = None
_CACHE = {}


# ------------------------------------------------------------------- builder
def _build():
    nc = bacc.Bacc("TRN2", target_bir_lowering=False, debug=False,
                   num_devices=NC)

    # pair-packed A^T shard: [128, j(47), t(2), s(1536)] so each DMA line is
    # 3072 B contiguous and each [128, 2, 512] slice is a DoubleRow moving AP
    at_d = nc.dram_tensor("at", [128, KP * 2 * SH], FP8, kind="ExternalInput")
    xs_d = nc.dram_tensor("xs", [128, KC * F], FP8, kind="ExternalInput")
    xt_d = nc.dram_tensor("xt", [F, SH], BF16, kind="ExternalInput")
    rr_d = nc.dram_tensor("rr", [1, SH], F32, kind="ExternalInput")
    w1t_d = nc.dram_tensor("w1t", [F, H], BF16, kind="ExternalInput")
    w1b_d = nc.dram_tensor("w1b", [F, H], BF16, kind="ExternalInput")
    w2t_d = nc.dram_tensor("w2t", [H, H], BF16, kind="ExternalInput")
    w2b_d = nc.dram_tensor("w2b", [H, H], BF16, kind="ExternalInput")
    w3t_d = nc.dram_tensor("w3t", [H, H], BF16, kind="ExternalInput")
    w3b_d = nc.dram_tensor("w3b", [H, H], BF16, kind="ExternalInput")
    wf1_d = nc.dram_tensor("wf1", [H, 2 * H], F32, kind="ExternalInput")
    wf2_d = nc.dram_tensor("wf2", [2 * H, 1], F32, kind="ExternalInput")
    b1_d = nc.dram_tensor("b1", [1, H], BF16, kind="ExternalInput")
    b2_d = nc.dram_tensor("b2", [1, H], BF16, kind="ExternalInput")
    b3_d = nc.dram_tensor("b3", [1, H], BF16, kind="ExternalInput")
    bf1_d = nc.dram_tensor("bf1", [2 * H, 1], F32, kind="ExternalInput")
    bf2_d = nc.dram_tensor("bf2", [1, 1], F32, kind="ExternalInput")
    i32_d = nc.dram_tensor("i32", [32, 32], BF16, kind="ExternalInput")
    out_d = nc.dram_tensor("out", [1, 1], F32, kind="ExternalOutput")

    # partition-major gather buffers: per-core block is [128, KG*H] with
    # contiguous per-partition lines on both the send and receive side
    ag_in = [nc.dram_tensor(f"ag_in{l}", [128, KG * H], FP8) for l in range(2)]
    ag_out = [nc.dram_tensor(f"ag_out{l}", [NC * 128, KG * H], FP8,
                             addr_space="Shared") for l in range(2)]
    ar_in = nc.dram_tensor("ar_in", [H, 1], F32)
    ar_out = nc.dram_tensor("ar_out", [H, 1], F32, addr_space="Shared")
    dmy_in = nc.dram_tensor("dmy_in", [1, 1], F32)
    dmy_out = nc.dram_tensor("dmy_out", [NC, 1], F32, addr_space="Shared")
    rg = [list(range(NC))]

    with tile.TileContext(nc) as tc:
        with (
            tc.tile_pool(name="const", bufs=1) as constp,
            tc.tile_pool(name="abig", bufs=1) as abigp,
            tc.tile_pool(name="xstat", bufs=1) as xstatp,
            tc.tile_pool(name="hstat", bufs=1) as hstatp,
            tc.tile_pool(name="hT", bufs=2) as hTp,
            tc.tile_pool(name="hnat", bufs=1) as hnatp,
            tc.tile_pool(name="rcb", bufs=1) as rcbp,
            tc.tile_pool(name="p3", bufs=3) as p3,
            tc.tile_pool(name="p2", bufs=2) as p2,
            tc.tile_pool(name="agg_ps", bufs=3, space=bass.MemorySpace.PSUM) as agg_ps,
            tc.tile_pool(name="z_ps", bufs=3, space=bass.MemorySpace.PSUM) as z_ps,
            tc.tile_pool(name="t_ps", bufs=2, space=bass.MemorySpace.PSUM) as t_ps,
        ):
            # first x-stationary group before everything else on gpsimd
            xs = xstatp.tile([128, KC, F], FP8)
            xs_r = xs_d.ap().rearrange("p (k f) -> p k f", f=F)
            nc.gpsimd.dma_start(xs[:, 0:8, :], xs_r[:, 0:8, :])

            # resident pair-packed A^T; stream the whole shard on sync queue
            at4 = abigp.tile([128, KP, 2, SH], FP8)
            at_r = at_d.ap().rearrange("p (j t s) -> p j t s", t=2, s=SH)
            for j in range(KP):
                nc.sync.dma_start(at4[:, j, :, :], at_r[:, j, :, :])

            # ---- constants
            def cload(dram, shape, dt=F32):
                t = constp.tile(shape, dt, tag=dram.name)
                nc.gpsimd.dma_start(t[:], dram[:, :])
                return t

            w1t = cload(w1t_d, [F, H], BF16)
            w1b = cload(w1b_d, [F, H], BF16)
            w2t = cload(w2t_d, [H, H], BF16)
            w2b = cload(w2b_d, [H, H], BF16)
            w3t = cload(w3t_d, [H, H], BF16)
            w3b = cload(w3b_d, [H, H], BF16)
            wf1 = cload(wf1_d, [H, 2 * H])
            wf2 = cload(wf2_d, [2 * H, 1])
            b1 = cload(b1_d, [1, H], BF16)
            b2 = cload(b2_d, [1, H], BF16)
            b3 = cload(b3_d, [1, H], BF16)
            bf1 = cload(bf1_d, [2 * H, 1])
            bf2 = cload(bf2_d, [1, 1])
            i32 = cload(i32_d, [32, 32], BF16)
            xt = cload(xt_d, [F, SH], BF16)
            rrow = cload(rr_d, [1, SH])

            ones_m = constp.tile([H, H], BF16, tag="ones_m")
            nc.gpsimd.memset(ones_m[:], 1.0)
            ones1 = constp.tile([1, 128], F32, tag="ones1")
            nc.gpsimd.memset(ones1[:], 1.0)
            onesb = constp.tile([1, 512], BF16, tag="onesb")
            nc.gpsimd.memset(onesb[:], 1.0)
            epsr = constp.tile([1, H], BF16, tag="epsr")
            nc.gpsimd.memset(epsr[:], 1e-12)

            # remaining x-stationary groups
            for g in range(8, KC, 8):
                ge = min(g + 8, KC)
                nc.gpsimd.dma_start(xs[:, g:ge, :], xs_r[:, g:ge, :])

            # broadcast 1/deg row to all 128 partitions via ones-matmul
            rc = rcbp.tile([F, SH], F32)
            for m0, mw in MT:
                prc = agg_ps.tile([F, 512], F32, tag="pagg", name=f"prc{m0}")
                nc.tensor.matmul(prc[:, :mw], ones1[:, :],
                                 rrow[:, m0:m0 + mw], start=True, stop=True)
                nc.vector.tensor_copy(rc[:, m0:m0 + mw], prc[:, :mw])

            # pre-wake the collective engines on all cores: the first
            # collective pays ~11us of CC firmware wake-up; absorb it here
            # under the A stream, off the critical path
            nc.gpsimd.dma_start(dmy_in[:, :], rc[0:1, 0:1])
            nc.gpsimd.collective_compute(
                "AllGather", mybir.AluOpType.bypass, replica_groups=rg,
                ins=[dmy_in.ap().opt()], outs=[dmy_out.ap().opt()])

            def layer(li, fl, h_stat, hT, wtop, wbot, b):
                """one SageConv layer; returns hT_next [H, SH] bf16 tile."""
                hTn = hTp.tile([H, SH], BF16, tag="hTn")
                # big aggregation matmul in fp8 DoubleRow: pair-outer, three
                # 512-col matmuls accumulate into three psum banks. A is
                # resident in SBUF (streamed from HBM during layer 0 only).
                paggs = []
                for _mi in range(len(MT)):
                    paggs.append(agg_ps.tile([fl, 512], F32, tag="pagg",
                                             name=f"pagg{li}_{_mi}"))
                TL = 4   # m-major tail: lets m-tile 0's post chain start
                for j in range(KP - TL):
                    hj = h_stat[:, 2 * j:2 * j + 2, :]
                    for mi, (m0, mw) in enumerate(MT):
                        nc.tensor.matmul(
                            paggs[mi][:, :mw], hj, at4[:, j, :, m0:m0 + mw],
                            start=(j == 0), stop=False, perf_mode=DR)
                for mi, (m0, mw) in enumerate(MT):
                    for j in range(KP - TL, KP):
                        hj = h_stat[:, 2 * j:2 * j + 2, :]
                        nc.tensor.matmul(
                            paggs[mi][:, :mw], hj, at4[:, j, :, m0:m0 + mw],
                            start=False, stop=(j == KP - 1), perf_mode=DR)
                pzs, pbcs = [], []
                for mi, (m0, mw) in enumerate(MT):
                    pagg = paggs[mi]
                    # scaled aggregation (mean): agg^T * (1/deg) broadcast
                    aggs = p2.tile([F, 512], BF16, tag="aggs")
                    nc.vector.tensor_mul(
                        aggs[:fl, :mw], pagg[:fl, :mw], rc[:fl, m0:m0 + mw])
                    # z^T = Wtop^T h^T + Wbot^T agg^T + b, with the bias as a
                    # rank-1 matmul so no extra DVE pass is needed
                    pz = z_ps.tile([H, 512], F32, tag="pz")
                    nc.tensor.matmul(pz[:, :mw], wtop[:, :], hT[:, m0:m0 + mw],
                                     start=True, stop=False)
                    nc.tensor.matmul(pz[:, :mw], wbot[:, :], aggs[:fl, :mw],
                                     start=False, stop=False)
                    nc.tensor.matmul(pz[:, :mw], b[:, :], onesb[:, :mw],
                                     start=False, stop=True)
                    # row l2-norm over features (partition dim): sumsq via
                    # ones-matmul, +1e-12 folded in as a rank-1 term standing
                    # in for the reference's max(ss, 1e-12)
                    zb = p3.tile([H, 512], BF16, tag="zb")
                    nc.vector.tensor_copy(zb[:, :mw], pz[:, :mw])
                    sq = p2.tile([H, 512], BF16, tag="sq")
                    nc.vector.tensor_mul(sq[:, :mw], zb[:, :mw], zb[:, :mw])
                    pbc = agg_ps.tile([H, 512], F32, tag="pagg",
                                      name=f"pbc{li}_{mi}")
                    nc.tensor.matmul(pbc[:, :mw], ones_m[:, :], sq[:, :mw],
                                     start=True, stop=False)
                    nc.tensor.matmul(pbc[:, :mw], epsr[:, :], onesb[:, :mw],
                                     start=False, stop=True)
                    pzs.append(zb)
                    pbcs.append(pbc)
                srts = []
                for mi, (m0, mw) in enumerate(MT):
                    srt = p3.tile([H, 512], F32, tag="srt")
                    nc.scalar.sqrt(srt[:, :mw], pbcs[mi][:, :mw])
                    srts.append(srt)
                if li < 2:
                    hnat = hnatp.tile([128, NJ, H], FP8, tag="hnat",
                                      name=f"hnat{li}")
                else:
                    hnat = None
                for mi, (m0, mw) in enumerate(MT):
                    rn = p2.tile([H, 512], F32, tag="rn")
                    nc.vector.reciprocal_approx_fast(rn[:, :mw],
                                                     srts[mi][:, :mw])
                    zn = p2.tile([H, 512], BF16, tag="zn")
                    nc.vector.tensor_mul(zn[:, :mw], pzs[mi][:, :mw],
                                         rn[:, :mw])
                    nc.scalar.activation(hTn[:, m0:m0 + mw], zn[:, :mw],
                                         mybir.ActivationFunctionType.Tanh)
                    if hnat is not None:
                        for jj in range(4):
                            j = mi * 4 + jj
                            pt = t_ps.tile([128, H], BF16, tag="pt")
                            nc.tensor.transpose(
                                pt[:, :], hTn[:, j * 128:(j + 1) * 128],
                                i32[:, :])
                            nc.vector.tensor_copy(hnat[:, j, :], pt[:, :])
                if hnat is not None:
                    nc.gpsimd.dma_start(
                        ag_in[li].ap().rearrange(
                            "p (j f) -> p j f", f=H)[:, :, :],
                        hnat[:, :, :])
                if li == 2:
                    return hTn, None
                nc.gpsimd.collective_compute(
                    "AllGather", mybir.AluOpType.bypass, replica_groups=rg,
                    ins=[ag_in[li].ap().opt()], outs=[ag_out[li].ap().opt()])
                if WARM > 0:
                    # optional PE keep-warm chain through the collective
                    pw = z_ps.tile([H, 512], F32, tag="pz", name=f"warm{li}")
                    for dmy in range(WARM):
                        nc.tensor.matmul(pw[:, :], at4[:, 0, 0, 0:H],
                                         at4[:, 0, 1, 0:512],
                                         start=(dmy == 0),
                                         stop=(dmy == WARM - 1))
                # gathered h in partition-major per-core blocks: one
                # contiguous DMA per peer core (384 B lines)
                h_stat_n = hstatp.tile([128, NC * KG, H], FP8, tag="hstat",
                                       name=f"hstat{li}")
                agor = ag_out[li].ap().rearrange("(c p) (j f) -> c p j f",
                                                 p=128, f=H)
                for c in range(NC):
                    nc.gpsimd.dma_start(h_stat_n[:, c * KG:(c + 1) * KG, :],
                                        agor[c, :, :, :])
                return hTn, h_stat_n

            hT1, hs1 = layer(0, F, xs, xt, w1t, w1b, b1)
            hT2, hs2 = layer(1, H, hs1, hT1, w2t, w2b, b2)
            hT3, _ = layer(2, H, hs2, hT2, w3t, w3b, b3)

            # global sum pool over this shard's nodes (padded nodes are 0);
            # reduced per m-slice so it overlaps the L3 post pipeline
            prt = p3.tile([H, 3], F32, tag="prt")
            for mi, (m0, mw) in enumerate(MT):
                nc.vector.reduce_sum(prt[:, mi:mi + 1], hT3[:, m0:m0 + mw],
                                     axis=mybir.AxisListType.X)
            pT = p2.tile([H, 1], F32, tag="pT")
            nc.vector.reduce_sum(pT[:, :], prt[:, :], axis=mybir.AxisListType.X)
            nc.gpsimd.dma_start(ar_in[:, :], pT[:])
            nc.gpsimd.collective_compute(
                "AllReduce", mybir.AluOpType.add, replica_groups=rg,
                ins=[ar_in.ap().opt()], outs=[ar_out.ap().opt()])
            pS = p2.tile([H, 1], F32, tag="pS")
            nc.gpsimd.dma_start(pS[:], ar_out[:, :])

            # final MLP (redundant on every core)
            pq = z_ps.tile([2 * H, 1], F32, tag="pz")
            nc.tensor.matmul(pq[:, :], wf1[:, :], pS[:, :], start=True, stop=True)
            q = p2.tile([2 * H, 1], F32, tag="q")
            nc.scalar.activation(q[:, :], pq[:, :],
                                 mybir.ActivationFunctionType.Tanh,
                                 bias=bf1[:])
            po = z_ps.tile([1, 1], F32, tag="pz")
            nc.tensor.matmul(po[:, :], wf2[:, :], q[:, :], start=True, stop=True)
            ob = p2.tile([1, 1], F32, tag="ob")
            nc.vector.tensor_scalar_add(ob[:, :], po[:, :], bf2[:])
            nc.gpsimd.dma_start(out_d[:, :], ob[:])

    nc.compile()
    return nc


# ---------------------------------------------------------------- host prep
def _prep(inputs):
    x = np.asarray(inputs["x"], np.float32)
    a = np.asarray(inputs["a"], np.float32)
    diag = np.diagonal(a).copy()
    add = (np.abs(diag) < TOL).astype(np.float32)
    deg = a.sum(axis=1) + add          # row sums of a_hat
    recip = np.ones(SH, np.float32)

    x_pad = np.zeros((NR, F), np.float32)
    x_pad[:N] = x
    # partition-major x chunks: xs[p, k, f] = x[k*128 + p, f]
    xs = np.ascontiguousarray(
        x_pad.reshape(KC, 128, F).transpose(1, 0, 2)).reshape(128, KC * F)
    xs = xs.astype(NP_FP8)

    w1 = np.asarray(inputs["W1"], np.float32)
    common = {
        "xs": xs,
        "w1t": w1[:F].astype(NP_BF16), "w1b": w1[F:].astype(NP_BF16),
        "w2t": np.asarray(inputs["W2"], np.float32)[:H].astype(NP_BF16),
        "w2b": np.asarray(inputs["W2"], np.float32)[H:].astype(NP_BF16),
        "w3t": np.asarray(inputs["W3"], np.float32)[:H].astype(NP_BF16),
        "w3b": np.asarray(inputs["W3"], np.float32)[H:].astype(NP_BF16),
        "wf1": np.asarray(inputs["Wf1"], np.float32),
        "wf2": np.asarray(inputs["Wf2"], np.float32),
        "b1": np.asarray(inputs["b1"], np.float32).reshape(1, H).astype(NP_BF16),
        "b2": np.asarray(inputs["b2"], np.float32).reshape(1, H).astype(NP_BF16),
        "b3": np.asarray(inputs["b3"], np.float32).reshape(1, H).astype(NP_BF16),
        "bf1": np.asarray(inputs["bf1"], np.float32).reshape(2 * H, 1),
        "bf2": np.asarray(inputs["bf2"], np.float32).reshape(1, 1),
        "i32": np.eye(32, dtype=NP_BF16),
    }

    in_maps = []
    for c in range(NC):
        r0 = c * SH
        r1 = min((c + 1) * SH, N)
        nrow = max(r1 - r0, 0)
        at = np.zeros((NR, SH), np.float32)
        if nrow > 0:
            at[:N, :nrow] = a[r0:r1].T          # [N(12000), nrow]
            # self-loops on approximately-zero diagonal entries
            idx = np.arange(nrow)
            gi = r0 + idx
            sel = add[gi] > 0
            at[gi[sel], idx[sel]] = a[gi[sel], gi[sel]] + 1.0
        # pair-packed fp8: at4[p, j, t, s] = at[(2j+t)*128 + p, s]
        at4 = np.ascontiguousarray(
            at.reshape(KP, 2, 128, SH).transpose(2, 0, 1, 3)
        ).reshape(128, KP * 2 * SH).astype(NP_FP8)
        xt = np.zeros((F, SH), NP_BF16)
        if nrow > 0:
            xt[:, :nrow] = x[r0:r1].T.astype(NP_BF16)
        rr = recip.copy()
        if nrow > 0:
            rr[:nrow] = 1.0 / deg[r0:r1]
        m = dict(common)
        m.update({"at": at4, "xt": xt, "rr": rr.reshape(1, SH)})
        in_maps.append(m)
    return in_maps


# -------------------------------------------------------------------- kernel
def kernel(**inputs):
    global LAST_EXEC_NS
    if "nc" not in _CACHE:
        _CACHE["nc"] = _build()
    nc = _CACHE["nc"]
    in_maps = _prep(inputs)
    res = run_bass_kernel_spmd(nc, in_maps, core_ids=list(range(NC)))
    LAST_EXEC_NS = res.exec_time_ns
    return np.asarray(res.results[0]["out"], np.float32).reshape(1, 1)
